# revision 22
# baseline (speedup 1.0000x reference)
"""GQA causal self-attention (dense transformer block) for 8x Trainium2 cores.

Reference semantics (B=8, T=1024, C=2048, H=16, hd=128, 4 KV heads, GQA r=4):
    q = rope(x @ wq), k = rope(tile(x @ wk)), v = tile(x @ wv)
    out = softmax_causal(q k^T / sqrt(hd)) v @ wo

Sharding: data-parallel over batch — one batch element per NeuronCore.

Per-core kernel layout (single batch element [T, C]):
  - host passes xT = x.T [C, T] bf16 so the contraction dim C sits on SBUF
    partitions for the projection matmuls
  - qT_h [hd, T] = wq_h^T @ xT (stationary = wq chunk, moving = xT slab);
    NeoX RoPE applied in [hd, T] layout on the vector engine (the rotate-half
    becomes a partition-half swap, done with cross-partition-window reads)
  - S^T[k, q] = kropeT.T @ qropeT per (ki, q >= ki) — causal upper blocks only
  - expS = exp(S^T * 1/sqrt(hd)) on the scalar engine (PSUM -> SBUF bf16),
    diagonal blocks masked with a triu 0/1 multiply
  - y[q, hd] via stationary = expS^T tile, moving = [v_g | ones] (the ones
    column accumulates the softmax denominator into PSUM column 128)
  - per-partition normalize (reciprocal + tensor_scalar_mul), PE transpose to
    yT, then out = yT.T @ wo accumulated over channel chunks -> [T, C] fp32
"""

import numpy as np
import ml_dtypes

BF16 = ml_dtypes.bfloat16

T = 1024
C = 2048
H = 16
HD = 128
KV = 4
CKV = 512
TT = T // 128
CCH = C // 128
SCALE = 1.0 / np.sqrt(128.0)
N_CORES = 8

_NC = None


def _build_nc():
    from contextlib import ExitStack

    import concourse.tile as tile
    from concourse import bacc, mybir

    f32 = mybir.dt.float32
    bf16 = mybir.dt.bfloat16
    Exp = mybir.ActivationFunctionType.Exp
    mult = mybir.AluOpType.mult

    nc = bacc.Bacc("TRN2", target_bir_lowering=False, debug=False,
                   num_devices=N_CORES)

    d_xt = nc.dram_tensor("xt", [C, T], bf16, kind="ExternalInput").ap()
    d_wq = nc.dram_tensor("wq", [C, C], bf16, kind="ExternalInput").ap()
    d_wk = nc.dram_tensor("wk", [C, CKV], bf16, kind="ExternalInput").ap()
    d_wv = nc.dram_tensor("wv", [C, CKV], bf16, kind="ExternalInput").ap()
    d_wo = nc.dram_tensor("wo", [C, C], bf16, kind="ExternalInput").ap()
    d_cos = nc.dram_tensor("cosT", [HD, T], f32, kind="ExternalInput").ap()
    d_sin = nc.dram_tensor("sinT", [HD, T], f32, kind="ExternalInput").ap()
    d_tri = nc.dram_tensor("tri", [128, 128], bf16, kind="ExternalInput").ap()
    d_id = nc.dram_tensor("ident", [128, 128], bf16, kind="ExternalInput").ap()
    d_out = nc.dram_tensor("out", [T, C], f32, kind="ExternalOutput").ap()

    with tile.TileContext(nc) as tc:
        with ExitStack() as ctx:
            consts = ctx.enter_context(tc.tile_pool(name="consts", bufs=1))
            xtp = ctx.enter_context(tc.tile_pool(name="xtp", bufs=8))
            wqp = ctx.enter_context(tc.tile_pool(name="wqp", bufs=6))
            wkvp = ctx.enter_context(tc.tile_pool(name="wkvp", bufs=5))
            wop = ctx.enter_context(tc.tile_pool(name="wop", bufs=6))
            qrp = ctx.enter_context(tc.tile_pool(name="qrp", bufs=4))
            krp = ctx.enter_context(tc.tile_pool(name="krp", bufs=4))
            vap = ctx.enter_context(tc.tile_pool(name="vap", bufs=32))
            expp = ctx.enter_context(tc.tile_pool(name="expp", bufs=2))
            spp = ctx.enter_context(tc.tile_pool(name="spp", bufs=3))
            yp = ctx.enter_context(tc.tile_pool(name="yp", bufs=4))
            rp = ctx.enter_context(tc.tile_pool(name="rp", bufs=8))
            ytp = ctx.enter_context(tc.tile_pool(name="ytp", bufs=16))
            outp = ctx.enter_context(tc.tile_pool(name="outp", bufs=3))
            ppsum = ctx.enter_context(
                tc.tile_pool(name="ppsum", bufs=2, space="PSUM"))
            spsum = ctx.enter_context(
                tc.tile_pool(name="spsum", bufs=3, space="PSUM"))
            ypsum = ctx.enter_context(
                tc.tile_pool(name="ypsum", bufs=2, space="PSUM"))
            tpsum = ctx.enter_context(
                tc.tile_pool(name="tpsum", bufs=1, space="PSUM"))

            # Grouped loads: one DMA per 4 channel-chunks (rearranged AP) so
            # the serial ~0.6us/issue HWDGE cost stops gating kernel start.
            # The critical-path loads alternate between the two HWDGE rings
            # (sync/scalar) to overlap issue.
            d_xt_r = d_xt.rearrange("(n p) m -> p n m", p=128)
            d_wk_r = d_wk.rearrange("(n p) m -> p n m", p=128)
            d_wv_r = d_wv.rearrange("(n p) m -> p n m", p=128)
            d_wq_r = d_wq.rearrange("(n p) m -> p n m", p=128)
            d_wo_r = d_wo.rearrange("(n p) m -> p n m", p=128)

            id_sb = consts.tile([128, 128], bf16, tag="ident")
            nc.scalar.dma_start(out=id_sb, in_=d_id)

            wkg = []
            xtg = {}
            # First chunk-pair split out so the k-proj chain starts sooner.
            wkg0 = wkvp.tile([128, 4, CKV], bf16, tag="wkv", name="wkg0")
            nc.scalar.dma_start(out=wkg0[:, 0:2, :], in_=d_wk_r[:, 0:2, :])
            xtg0 = xtp.tile([128, 4, 512], bf16, tag="xt", name="xtg0_0")
            nc.sync.dma_start(out=xtg0[:, 0:2, :], in_=d_xt_r[:, 0:2, 0:512])
            nc.scalar.dma_start(out=wkg0[:, 2:4, :], in_=d_wk_r[:, 2:4, :])
            nc.sync.dma_start(out=xtg0[:, 2:4, :], in_=d_xt_r[:, 2:4, 0:512])
            wkg.append(wkg0)
            xtg[(0, 0)] = xtg0
            for j in range(1, 4):
                t_ = wkvp.tile([128, 4, CKV], bf16, tag="wkv", name=f"wkg{j}")
                nc.scalar.dma_start(out=t_,
                                    in_=d_wk_r[:, j * 4:(j + 1) * 4, :])
                wkg.append(t_)
                t2 = xtp.tile([128, 4, 512], bf16, tag="xt", name=f"xtg{j}_0")
                nc.sync.dma_start(
                    out=t2, in_=d_xt_r[:, j * 4:(j + 1) * 4, 0:512])
                xtg[(j, 0)] = t2

            # Warm the PE (HAM clock-gate) while the first activations
            # stream in. Must be real matmuls — transpose-mode does not
            # count as PE-busy for HAM. The operand comes from a memset so
            # warmup starts right after the preamble, not after a DMA.
            wsb = consts.tile([128, 128], bf16, tag="warm_sb")
            nc.vector.memset(wsb, 0.0)
            wtp = tpsum.tile([128, 128], f32, tag="tp", name="warm")
            for _ in range(40):
                nc.tensor.matmul(wtp, wsb, wsb, start=True, stop=True)

            cos_sb = consts.tile([128, T], f32, tag="cos")
            nc.gpsimd.dma_start(out=cos_sb, in_=d_cos)
            sin_sb = consts.tile([128, T], f32, tag="sin")
            nc.gpsimd.dma_start(out=sin_sb, in_=d_sin)
            tri_sb = consts.tile([128, 128], bf16, tag="tri")
            nc.gpsimd.dma_start(out=tri_sb, in_=d_tri)
            for j in range(4):
                t2 = xtp.tile([128, 4, 512], bf16, tag="xt", name=f"xtg{j}_1")
                nc.sync.dma_start(
                    out=t2, in_=d_xt_r[:, j * 4:(j + 1) * 4, 512:T])
                xtg[(j, 1)] = t2

            def xt_slab(c, s):
                return xtg[(c // 4, s)][:, c % 4, :]

            def xt_stat(c, ti):
                return xtg[(c // 4, ti // 4)][:, c % 4,
                                              (ti % 4) * 128:(ti % 4 + 1) * 128]

            def wk_stat(c, g):
                return wkg[c // 4][:, c % 4, g * 128:(g + 1) * 128]

            def rope_into(dst_bf16, ps, s):
                sl = slice(s * 512, (s + 1) * 512)
                sp = spp.tile([128, 512], f32, tag="sp", name=f"sp{s}")
                nc.vector.scalar_tensor_tensor(
                    out=sp[0:64, :], in0=ps[64:128, :], scalar=-1.0,
                    in1=sin_sb[0:64, sl], op0=mult, op1=mult)
                nc.vector.scalar_tensor_tensor(
                    out=sp[64:128, :], in0=ps[0:64, :], scalar=1.0,
                    in1=sin_sb[64:128, sl], op0=mult, op1=mult)
                cp = spp.tile([128, 512], f32, tag="cp", name=f"cp{s}")
                nc.vector.tensor_mul(cp, ps, cos_sb[:, sl])
                nc.vector.tensor_add(dst_bf16[:, sl], cp, sp)

            krope = []
            for g in range(KV):
                kr = krp.tile([128, T], bf16, tag="kr", name=f"kr{g}")
                krope.append(kr)
            # Run k-proj chains in waves of 3 (PSUM pool depth), c-major
            # within a wave: each arriving xT group feeds 3 chains' matmuls,
            # so the DMA-paced prefill window does 3x the PE work.
            kchains = [(g, s) for s in range(2) for g in range(KV)]
            for wave in (kchains[0:2], kchains[2:4], kchains[4:6],
                         kchains[6:8]):
                kps = {}
                for (g, s) in wave:
                    kps[(g, s)] = ppsum.tile([128, 512], f32, tag="pp",
                                             name=f"pk{g}{s}")
                for c in range(CCH):
                    for (g, s) in wave:
                        nc.tensor.matmul(
                            kps[(g, s)], wk_stat(c, g), xt_slab(c, s),
                            start=(c == 0), stop=(c == CCH - 1))
                for (g, s) in wave:
                    rope_into(krope[g], kps[(g, s)], s)

            wq_state = {"sb": [], "slab": -1}
            qropes = {}

            def qproj(h):
                s4 = h // 4
                if s4 != wq_state["slab"]:
                    wq_state["slab"] = s4
                    wq_state["sb"] = []
                    for j in range(4):
                        t_ = wqp.tile([128, 4, 512], bf16, tag="wq",
                                      name=f"wq{s4}_{j}")
                        nc.sync.dma_start(
                            out=t_,
                            in_=d_wq_r[:, j * 4:(j + 1) * 4,
                                       s4 * 512:(s4 + 1) * 512])
                        wq_state["sb"].append(t_)
                hh = (h % 4) * 128
                qr = qrp.tile([128, T], bf16, tag="qr", name=f"qr{h}")
                for s in range(2):
                    ps = ppsum.tile([128, 512], f32, tag="pp", name=f"pq{h}{s}")
                    for c in range(CCH):
                        nc.tensor.matmul(
                            ps, wq_state["sb"][c // 4][:, c % 4, hh:hh + 128],
                            xt_slab(c, s),
                            start=(c == 0), stop=(c == CCH - 1))
                    rope_into(qr, ps, s)
                qropes[h] = qr

            def qproj_gen(h):
                s4 = h // 4
                if s4 != wq_state["slab"]:
                    wq_state["slab"] = s4
                    wq_state["sb"] = []
                    for j in range(4):
                        t_ = wqp.tile([128, 4, 512], bf16, tag="wq",
                                      name=f"wq{s4}_{j}")
                        nc.sync.dma_start(
                            out=t_,
                            in_=d_wq_r[:, j * 4:(j + 1) * 4,
                                       s4 * 512:(s4 + 1) * 512])
                        wq_state["sb"].append(t_)
                hh = (h % 4) * 128
                qr = qrp.tile([128, T], bf16, tag="qr", name=f"qr{h}")
                for s in range(2):
                    ps = ppsum.tile([128, 512], f32, tag="pp", name=f"pq{h}{s}")
                    for c in range(CCH):
                        nc.tensor.matmul(
                            ps, wq_state["sb"][c // 4][:, c % 4, hh:hh + 128],
                            xt_slab(c, s),
                            start=(c == 0), stop=(c == CCH - 1))
                        if c % 2 == 1:
                            yield
                    rope_into(qr, ps, s)
                qropes[h] = qr

            qproj(0)
            qproj(1)

            wvg = []
            for j in range(4):
                t_ = wkvp.tile([128, 4, CKV], bf16, tag="wkv", name=f"wvg{j}")
                nc.sync.dma_start(out=t_, in_=d_wv_r[:, j * 4:(j + 1) * 4, :])
                wvg.append(t_)
            vaug = {}
            for ti in range(TT):
                ps = ppsum.tile([128, CKV], f32, tag="pp", name=f"pv{ti}")
                for c in range(CCH):
                    nc.tensor.matmul(
                        ps, xt_stat(c, ti), wvg[c // 4][:, c % 4, :],
                        start=(c == 0), stop=(c == CCH - 1))
                for g in range(KV):
                    va = vap.tile([128, 132], bf16, tag="va", name=f"va{g}{ti}")
                    nc.vector.tensor_copy(va[:, 0:128],
                                          ps[:, g * 128:(g + 1) * 128])
                    nc.vector.memset(va[:, 128:129], 1.0)
                    vaug[(g, ti)] = va

            exp_store = {}
            yts_d = {}

            def s_gen(h):
                """S^T = kropeT.T @ qropeT, exp on ACT, diag mask."""
                g = h % KV
                qr = qropes.pop(h)
                kr = krope[g]
                ets = []
                for ki in range(TT):
                    width = T - ki * 128
                    et = expp.tile([128, width], bf16, tag=f"exp{ki}",
                                   name=f"et{h}_{ki}")
                    off = ki * 128
                    while off < T:
                        w = min(512, T - off)
                        sps = spsum.tile([128, 512], f32, tag="sps",
                                         name=f"sps{h}{ki}{off}")
                        nc.tensor.matmul(
                            sps[:, :w], kr[:, ki * 128:(ki + 1) * 128],
                            qr[:, off:off + w], start=True, stop=True)
                        nc.scalar.activation(
                            et[:, off - ki * 128:off - ki * 128 + w],
                            sps[:, :w], Exp, scale=float(SCALE))
                        yield
                        off += w
                    nc.vector.tensor_mul(et[:, 0:128], et[:, 0:128], tri_sb)
                    ets.append(et)
                exp_store[h] = ets

            def attv_gen(h):
                g = h % KV
                ets = exp_store.pop(h)
                yt = ytp.tile([128, T], bf16, tag="yt", name=f"yt{h}")
                for qi in range(TT):
                    ypt = ypsum.tile([128, 132], f32, tag="yps",
                                     name=f"yps{h}{qi}")
                    for ki in range(qi + 1):
                        nc.tensor.matmul(
                            ypt[:, 0:129],
                            ets[ki][:, (qi - ki) * 128:(qi - ki + 1) * 128],
                            vaug[(g, ki)][:, 0:129],
                            start=(ki == 0), stop=(ki == qi))
                        if ki % 2 == 1:
                            yield
                    r = rp.tile([128, 1], f32, tag="r", name=f"r{h}{qi}")
                    nc.vector.reciprocal(r, ypt[:, 128:129])
                    ysb = yp.tile([128, 128], bf16, tag="y", name=f"y{h}{qi}")
                    nc.vector.tensor_scalar_mul(ysb, ypt[:, 0:128], r)
                    tp = tpsum.tile([128, 128], bf16, tag="tp",
                                    name=f"tp{h}{qi}")
                    nc.tensor.transpose(tp, ysb, id_sb)
                    nc.vector.tensor_copy(yt[:, qi * 128:(qi + 1) * 128], tp)
                    yield
                yts_d[h] = yt

            def drive(gens):
                # Round-robin the emission streams so the PE program
                # interleaves S^T (ACT-paced), att@v, and q-proj matmuls —
                # the in-order PE always has non-stalled work queued.
                gens = list(gens)
                while gens:
                    keep = []
                    for g_ in gens:
                        try:
                            next(g_)
                            keep.append(g_)
                        except StopIteration:
                            pass
                    gens = keep

            for it in range(H + 1):
                active = []
                if it >= 1:
                    active.append(attv_gen(it - 1))
                if it + 2 < H:
                    active.append(qproj_gen(it + 2))
                if it < H:
                    active.append(s_gen(it))
                drive(active)
            yts = [yts_d[h] for h in range(H)]

            for n in range(4):
                wo_sb = []
                for j in range(4):
                    t_ = wop.tile([128, 4, 512], bf16, tag="wo",
                                  name=f"wo{n}_{j}")
                    nc.sync.dma_start(
                        out=t_,
                        in_=d_wo_r[:, j * 4:(j + 1) * 4,
                                   n * 512:(n + 1) * 512])
                    wo_sb.append(t_)
                for ti in range(TT):
                    ps = ppsum.tile([128, 512], f32, tag="pp",
                                    name=f"po{n}{ti}")
                    for c in range(CCH):
                        nc.tensor.matmul(
                            ps, yts[c][:, ti * 128:(ti + 1) * 128],
                            wo_sb[c // 4][:, c % 4, :],
                            start=(c == 0), stop=(c == CCH - 1))
                    osb = outp.tile([128, 512], f32, tag="o", name=f"o{n}{ti}")
                    nc.vector.tensor_copy(osb, ps)
                    nc.sync.dma_start(
                        out=d_out[ti * 128:(ti + 1) * 128,
                                  n * 512:(n + 1) * 512],
                        in_=osb)

    nc.compile()
    return nc


def _get_nc():
    global _NC
    if _NC is None:
        _NC = _build_nc()
    return _NC


def _host_prep(x, wq, wk, wv, wo, sin, cos):
    x = np.asarray(x, np.float32)
    wq_b = np.asarray(wq, np.float32).astype(BF16)
    wk_b = np.asarray(wk, np.float32).astype(BF16)
    wv_b = np.asarray(wv, np.float32).astype(BF16)
    wo_b = np.asarray(wo, np.float32).astype(BF16)
    cosT = np.ascontiguousarray(np.asarray(cos, np.float32).T)
    sinT = np.ascontiguousarray(np.asarray(sin, np.float32).T)
    tri = np.triu(np.ones((128, 128), np.float32)).astype(BF16)
    ident = np.eye(128, dtype=np.float32).astype(BF16)
    maps = []
    for b in range(N_CORES):
        xt = np.ascontiguousarray(x[b].T).astype(BF16)
        maps.append(dict(xt=xt, wq=wq_b, wk=wk_b, wv=wv_b, wo=wo_b,
                         cosT=cosT, sinT=sinT, tri=tri, ident=ident))
    return maps


def run_spmd(in_maps, trace=False, **kwargs):
    from concourse import bass_utils
    nc = _get_nc()
    return bass_utils.run_bass_kernel_spmd(
        nc, in_maps, core_ids=list(range(N_CORES)), trace=trace, **kwargs)


def kernel(x, wq, wk, wv, wo, sin, cos):
    in_maps = _host_prep(x, wq, wk, wv, wo, sin, cos)
    res = run_spmd(in_maps)
    out = np.stack([np.asarray(res.results[b]["out"], np.float32)
                    for b in range(N_CORES)], axis=0)
    return out


# revision 23
# speedup vs baseline: 1.0183x; 1.0183x over previous
"""GQA causal self-attention (dense transformer block) for 8x Trainium2 cores.

Reference semantics (B=8, T=1024, C=2048, H=16, hd=128, 4 KV heads, GQA r=4):
    q = rope(x @ wq), k = rope(tile(x @ wk)), v = tile(x @ wv)
    out = softmax_causal(q k^T / sqrt(hd)) v @ wo

Sharding: data-parallel over batch — one batch element per NeuronCore.

Per-core kernel layout (single batch element [T, C]):
  - host passes xT = x.T [C, T] bf16 so the contraction dim C sits on SBUF
    partitions for the projection matmuls
  - qT_h [hd, T] = wq_h^T @ xT (stationary = wq chunk, moving = xT slab);
    NeoX RoPE applied in [hd, T] layout on the vector engine (the rotate-half
    becomes a partition-half swap, done with cross-partition-window reads)
  - S^T[k, q] = kropeT.T @ qropeT per (ki, q >= ki) — causal upper blocks only
  - expS = exp(S^T * 1/sqrt(hd)) on the scalar engine (PSUM -> SBUF bf16),
    diagonal blocks masked with a triu 0/1 multiply
  - y[q, hd] via stationary = expS^T tile, moving = [v_g | ones] (the ones
    column accumulates the softmax denominator into PSUM column 128)
  - per-partition normalize (reciprocal + tensor_scalar_mul), PE transpose to
    yT, then out = yT.T @ wo accumulated over channel chunks -> [T, C] fp32
"""

import numpy as np
import ml_dtypes

BF16 = ml_dtypes.bfloat16

T = 1024
C = 2048
H = 16
HD = 128
KV = 4
CKV = 512
TT = T // 128
CCH = C // 128
SCALE = 1.0 / np.sqrt(128.0)
N_CORES = 8

_NC = None


def _build_nc():
    from contextlib import ExitStack

    import concourse.tile as tile
    from concourse import bacc, mybir

    f32 = mybir.dt.float32
    bf16 = mybir.dt.bfloat16
    Exp = mybir.ActivationFunctionType.Exp
    mult = mybir.AluOpType.mult

    nc = bacc.Bacc("TRN2", target_bir_lowering=False, debug=False,
                   num_devices=N_CORES)

    d_xt = nc.dram_tensor("xt", [C, T], bf16, kind="ExternalInput").ap()
    d_wq = nc.dram_tensor("wq", [C, C], bf16, kind="ExternalInput").ap()
    d_wk = nc.dram_tensor("wk", [C, CKV], bf16, kind="ExternalInput").ap()
    d_wv = nc.dram_tensor("wv", [C, CKV], bf16, kind="ExternalInput").ap()
    d_wo = nc.dram_tensor("wo", [C, C], bf16, kind="ExternalInput").ap()
    d_cos = nc.dram_tensor("cosT", [HD, T], f32, kind="ExternalInput").ap()
    d_sin = nc.dram_tensor("sinT", [HD, T], f32, kind="ExternalInput").ap()
    d_tri = nc.dram_tensor("tri", [128, 128], bf16, kind="ExternalInput").ap()
    d_id = nc.dram_tensor("ident", [128, 128], bf16, kind="ExternalInput").ap()
    d_out = nc.dram_tensor("out", [T, C], f32, kind="ExternalOutput").ap()

    with tile.TileContext(nc) as tc:
        with ExitStack() as ctx:
            consts = ctx.enter_context(tc.tile_pool(name="consts", bufs=1))
            xtp = ctx.enter_context(tc.tile_pool(name="xtp", bufs=8))
            wqp = ctx.enter_context(tc.tile_pool(name="wqp", bufs=6))
            wkvp = ctx.enter_context(tc.tile_pool(name="wkvp", bufs=5))
            wop = ctx.enter_context(tc.tile_pool(name="wop", bufs=6))
            qrp = ctx.enter_context(tc.tile_pool(name="qrp", bufs=4))
            krp = ctx.enter_context(tc.tile_pool(name="krp", bufs=4))
            vap = ctx.enter_context(tc.tile_pool(name="vap", bufs=32))
            expp = ctx.enter_context(tc.tile_pool(name="expp", bufs=2))
            spp = ctx.enter_context(tc.tile_pool(name="spp", bufs=3))
            yp = ctx.enter_context(tc.tile_pool(name="yp", bufs=4))
            rp = ctx.enter_context(tc.tile_pool(name="rp", bufs=8))
            ytp = ctx.enter_context(tc.tile_pool(name="ytp", bufs=16))
            outp = ctx.enter_context(tc.tile_pool(name="outp", bufs=3))
            ppsum = ctx.enter_context(
                tc.tile_pool(name="ppsum", bufs=3, space="PSUM"))
            spsum = ctx.enter_context(
                tc.tile_pool(name="spsum", bufs=2, space="PSUM"))
            ypsum = ctx.enter_context(
                tc.tile_pool(name="ypsum", bufs=2, space="PSUM"))
            tpsum = ctx.enter_context(
                tc.tile_pool(name="tpsum", bufs=1, space="PSUM"))

            # Grouped loads: one DMA per 4 channel-chunks (rearranged AP) so
            # the serial ~0.6us/issue HWDGE cost stops gating kernel start.
            # The critical-path loads alternate between the two HWDGE rings
            # (sync/scalar) to overlap issue.
            d_xt_r = d_xt.rearrange("(n p) m -> p n m", p=128)
            d_wk_r = d_wk.rearrange("(n p) m -> p n m", p=128)
            d_wv_r = d_wv.rearrange("(n p) m -> p n m", p=128)
            d_wq_r = d_wq.rearrange("(n p) m -> p n m", p=128)
            d_wo_r = d_wo.rearrange("(n p) m -> p n m", p=128)

            id_sb = consts.tile([128, 128], bf16, tag="ident")
            nc.scalar.dma_start(out=id_sb, in_=d_id)

            wkg = []
            xtg = {}
            # First chunk-pair split out so the k-proj chain starts sooner.
            wkg0 = wkvp.tile([128, 4, CKV], bf16, tag="wkv", name="wkg0")
            nc.scalar.dma_start(out=wkg0[:, 0:2, :], in_=d_wk_r[:, 0:2, :])
            xtg0 = xtp.tile([128, 4, 512], bf16, tag="xt", name="xtg0_0")
            nc.sync.dma_start(out=xtg0[:, 0:2, :], in_=d_xt_r[:, 0:2, 0:512])
            nc.scalar.dma_start(out=wkg0[:, 2:4, :], in_=d_wk_r[:, 2:4, :])
            nc.sync.dma_start(out=xtg0[:, 2:4, :], in_=d_xt_r[:, 2:4, 0:512])
            wkg.append(wkg0)
            xtg[(0, 0)] = xtg0
            for j in range(1, 4):
                t_ = wkvp.tile([128, 4, CKV], bf16, tag="wkv", name=f"wkg{j}")
                nc.scalar.dma_start(out=t_,
                                    in_=d_wk_r[:, j * 4:(j + 1) * 4, :])
                wkg.append(t_)
                t2 = xtp.tile([128, 4, 512], bf16, tag="xt", name=f"xtg{j}_0")
                nc.sync.dma_start(
                    out=t2, in_=d_xt_r[:, j * 4:(j + 1) * 4, 0:512])
                xtg[(j, 0)] = t2

            # Warm the PE (HAM clock-gate) while the first activations
            # stream in. Must be real matmuls — transpose-mode does not
            # count as PE-busy for HAM. The operand comes from a memset so
            # warmup starts right after the preamble, not after a DMA.
            wsb = consts.tile([128, 128], bf16, tag="warm_sb")
            nc.vector.memset(wsb, 0.0)
            wtp = tpsum.tile([128, 128], f32, tag="tp", name="warm")
            for _ in range(40):
                nc.tensor.matmul(wtp, wsb, wsb, start=True, stop=True)

            cos_sb = consts.tile([128, T], f32, tag="cos")
            nc.gpsimd.dma_start(out=cos_sb, in_=d_cos)
            sin_sb = consts.tile([128, T], f32, tag="sin")
            nc.gpsimd.dma_start(out=sin_sb, in_=d_sin)
            tri_sb = consts.tile([128, 128], bf16, tag="tri")
            nc.gpsimd.dma_start(out=tri_sb, in_=d_tri)
            for j in range(4):
                t2 = xtp.tile([128, 4, 512], bf16, tag="xt", name=f"xtg{j}_1")
                nc.sync.dma_start(
                    out=t2, in_=d_xt_r[:, j * 4:(j + 1) * 4, 512:T])
                xtg[(j, 1)] = t2

            def xt_slab(c, s):
                return xtg[(c // 4, s)][:, c % 4, :]

            def xt_stat(c, ti):
                return xtg[(c // 4, ti // 4)][:, c % 4,
                                              (ti % 4) * 128:(ti % 4 + 1) * 128]

            def wk_stat(c, g):
                return wkg[c // 4][:, c % 4, g * 128:(g + 1) * 128]

            def rope_into(dst_bf16, ps, s):
                sl = slice(s * 512, (s + 1) * 512)
                sp = spp.tile([128, 512], f32, tag="sp", name=f"sp{s}")
                nc.vector.scalar_tensor_tensor(
                    out=sp[0:64, :], in0=ps[64:128, :], scalar=-1.0,
                    in1=sin_sb[0:64, sl], op0=mult, op1=mult)
                nc.vector.scalar_tensor_tensor(
                    out=sp[64:128, :], in0=ps[0:64, :], scalar=1.0,
                    in1=sin_sb[64:128, sl], op0=mult, op1=mult)
                cp = spp.tile([128, 512], f32, tag="cp", name=f"cp{s}")
                nc.vector.tensor_mul(cp, ps, cos_sb[:, sl])
                nc.vector.tensor_add(dst_bf16[:, sl], cp, sp)

            krope = []
            for g in range(KV):
                kr = krp.tile([128, T], bf16, tag="kr", name=f"kr{g}")
                krope.append(kr)
            # Run k-proj chains in waves of 3 (PSUM pool depth), c-major
            # within a wave: each arriving xT group feeds 3 chains' matmuls,
            # so the DMA-paced prefill window does 3x the PE work.
            kchains = [(g, s) for s in range(2) for g in range(KV)]
            for wave in (kchains[0:3], kchains[3:6], kchains[6:8]):
                kps = {}
                for (g, s) in wave:
                    kps[(g, s)] = ppsum.tile([128, 512], f32, tag="pp",
                                             name=f"pk{g}{s}")
                for c in range(CCH):
                    for (g, s) in wave:
                        nc.tensor.matmul(
                            kps[(g, s)], wk_stat(c, g), xt_slab(c, s),
                            start=(c == 0), stop=(c == CCH - 1))
                for (g, s) in wave:
                    rope_into(krope[g], kps[(g, s)], s)

            wq_state = {"sb": [], "slab": -1}
            qropes = {}

            def qproj(h):
                s4 = h // 4
                if s4 != wq_state["slab"]:
                    wq_state["slab"] = s4
                    wq_state["sb"] = []
                    for j in range(4):
                        t_ = wqp.tile([128, 4, 512], bf16, tag="wq",
                                      name=f"wq{s4}_{j}")
                        nc.sync.dma_start(
                            out=t_,
                            in_=d_wq_r[:, j * 4:(j + 1) * 4,
                                       s4 * 512:(s4 + 1) * 512])
                        wq_state["sb"].append(t_)
                hh = (h % 4) * 128
                qr = qrp.tile([128, T], bf16, tag="qr", name=f"qr{h}")
                for s in range(2):
                    ps = ppsum.tile([128, 512], f32, tag="pp", name=f"pq{h}{s}")
                    for c in range(CCH):
                        nc.tensor.matmul(
                            ps, wq_state["sb"][c // 4][:, c % 4, hh:hh + 128],
                            xt_slab(c, s),
                            start=(c == 0), stop=(c == CCH - 1))
                    rope_into(qr, ps, s)
                qropes[h] = qr

            def qproj_gen(h):
                s4 = h // 4
                if s4 != wq_state["slab"]:
                    wq_state["slab"] = s4
                    wq_state["sb"] = []
                    for j in range(4):
                        t_ = wqp.tile([128, 4, 512], bf16, tag="wq",
                                      name=f"wq{s4}_{j}")
                        nc.sync.dma_start(
                            out=t_,
                            in_=d_wq_r[:, j * 4:(j + 1) * 4,
                                       s4 * 512:(s4 + 1) * 512])
                        wq_state["sb"].append(t_)
                hh = (h % 4) * 128
                qr = qrp.tile([128, T], bf16, tag="qr", name=f"qr{h}")
                for s in range(2):
                    ps = ppsum.tile([128, 512], f32, tag="pp", name=f"pq{h}{s}")
                    for c in range(CCH):
                        nc.tensor.matmul(
                            ps, wq_state["sb"][c // 4][:, c % 4, hh:hh + 128],
                            xt_slab(c, s),
                            start=(c == 0), stop=(c == CCH - 1))
                        if c % 2 == 1:
                            yield
                    rope_into(qr, ps, s)
                qropes[h] = qr

            qproj(0)
            qproj(1)

            wvg = []
            for j in range(4):
                t_ = wkvp.tile([128, 4, CKV], bf16, tag="wkv", name=f"wvg{j}")
                nc.sync.dma_start(out=t_, in_=d_wv_r[:, j * 4:(j + 1) * 4, :])
                wvg.append(t_)
            vaug = {}
            for ti in range(TT):
                ps = ppsum.tile([128, CKV], f32, tag="pp", name=f"pv{ti}")
                for c in range(CCH):
                    nc.tensor.matmul(
                        ps, xt_stat(c, ti), wvg[c // 4][:, c % 4, :],
                        start=(c == 0), stop=(c == CCH - 1))
                for g in range(KV):
                    va = vap.tile([128, 132], bf16, tag="va", name=f"va{g}{ti}")
                    nc.vector.tensor_copy(va[:, 0:128],
                                          ps[:, g * 128:(g + 1) * 128])
                    nc.vector.memset(va[:, 128:129], 1.0)
                    vaug[(g, ti)] = va

            exp_store = {}
            yts_d = {}

            def s_gen(h):
                """S^T = kropeT.T @ qropeT, exp on ACT, diag mask."""
                g = h % KV
                qr = qropes.pop(h)
                kr = krope[g]
                ets = []
                for ki in range(TT):
                    width = T - ki * 128
                    et = expp.tile([128, width], bf16, tag=f"exp{ki}",
                                   name=f"et{h}_{ki}")
                    off = ki * 128
                    while off < T:
                        w = min(512, T - off)
                        sps = spsum.tile([128, 512], f32, tag="sps",
                                         name=f"sps{h}{ki}{off}")
                        nc.tensor.matmul(
                            sps[:, :w], kr[:, ki * 128:(ki + 1) * 128],
                            qr[:, off:off + w], start=True, stop=True)
                        nc.scalar.activation(
                            et[:, off - ki * 128:off - ki * 128 + w],
                            sps[:, :w], Exp, scale=float(SCALE))
                        yield
                        off += w
                    nc.vector.tensor_mul(et[:, 0:128], et[:, 0:128], tri_sb)
                    ets.append(et)
                exp_store[h] = ets

            def attv_gen(h):
                g = h % KV
                ets = exp_store.pop(h)
                yt = ytp.tile([128, T], bf16, tag="yt", name=f"yt{h}")
                for qi in range(TT):
                    ypt = ypsum.tile([128, 132], f32, tag="yps",
                                     name=f"yps{h}{qi}")
                    for ki in range(qi + 1):
                        nc.tensor.matmul(
                            ypt[:, 0:129],
                            ets[ki][:, (qi - ki) * 128:(qi - ki + 1) * 128],
                            vaug[(g, ki)][:, 0:129],
                            start=(ki == 0), stop=(ki == qi))
                        if ki % 2 == 1:
                            yield
                    r = rp.tile([128, 1], f32, tag="r", name=f"r{h}{qi}")
                    nc.vector.reciprocal(r, ypt[:, 128:129])
                    ysb = yp.tile([128, 128], bf16, tag="y", name=f"y{h}{qi}")
                    nc.vector.tensor_scalar_mul(ysb, ypt[:, 0:128], r)
                    tp = tpsum.tile([128, 128], bf16, tag="tp",
                                    name=f"tp{h}{qi}")
                    nc.tensor.transpose(tp, ysb, id_sb)
                    nc.vector.tensor_copy(yt[:, qi * 128:(qi + 1) * 128], tp)
                    yield
                yts_d[h] = yt

            def drive(gens):
                # Round-robin the emission streams so the PE program
                # interleaves S^T (ACT-paced), att@v, and q-proj matmuls —
                # the in-order PE always has non-stalled work queued.
                gens = list(gens)
                while gens:
                    keep = []
                    for g_ in gens:
                        try:
                            next(g_)
                            keep.append(g_)
                        except StopIteration:
                            pass
                    gens = keep

            for it in range(H + 1):
                active = []
                if it >= 1:
                    active.append(attv_gen(it - 1))
                if it + 2 < H:
                    active.append(qproj_gen(it + 2))
                if it < H:
                    active.append(s_gen(it))
                drive(active)
            yts = [yts_d[h] for h in range(H)]

            for n in range(4):
                wo_sb = []
                for j in range(4):
                    t_ = wop.tile([128, 4, 512], bf16, tag="wo",
                                  name=f"wo{n}_{j}")
                    nc.sync.dma_start(
                        out=t_,
                        in_=d_wo_r[:, j * 4:(j + 1) * 4,
                                   n * 512:(n + 1) * 512])
                    wo_sb.append(t_)
                for ti in range(TT):
                    ps = ppsum.tile([128, 512], f32, tag="pp",
                                    name=f"po{n}{ti}")
                    for c in range(CCH):
                        nc.tensor.matmul(
                            ps, yts[c][:, ti * 128:(ti + 1) * 128],
                            wo_sb[c // 4][:, c % 4, :],
                            start=(c == 0), stop=(c == CCH - 1))
                    osb = outp.tile([128, 512], f32, tag="o", name=f"o{n}{ti}")
                    nc.vector.tensor_copy(osb, ps)
                    nc.sync.dma_start(
                        out=d_out[ti * 128:(ti + 1) * 128,
                                  n * 512:(n + 1) * 512],
                        in_=osb)

    nc.compile()
    return nc


def _get_nc():
    global _NC
    if _NC is None:
        _NC = _build_nc()
    return _NC


def _host_prep(x, wq, wk, wv, wo, sin, cos):
    x = np.asarray(x, np.float32)
    wq_b = np.asarray(wq, np.float32).astype(BF16)
    wk_b = np.asarray(wk, np.float32).astype(BF16)
    wv_b = np.asarray(wv, np.float32).astype(BF16)
    wo_b = np.asarray(wo, np.float32).astype(BF16)
    cosT = np.ascontiguousarray(np.asarray(cos, np.float32).T)
    sinT = np.ascontiguousarray(np.asarray(sin, np.float32).T)
    tri = np.triu(np.ones((128, 128), np.float32)).astype(BF16)
    ident = np.eye(128, dtype=np.float32).astype(BF16)
    maps = []
    for b in range(N_CORES):
        xt = np.ascontiguousarray(x[b].T).astype(BF16)
        maps.append(dict(xt=xt, wq=wq_b, wk=wk_b, wv=wv_b, wo=wo_b,
                         cosT=cosT, sinT=sinT, tri=tri, ident=ident))
    return maps


def run_spmd(in_maps, trace=False, **kwargs):
    from concourse import bass_utils
    nc = _get_nc()
    return bass_utils.run_bass_kernel_spmd(
        nc, in_maps, core_ids=list(range(N_CORES)), trace=trace, **kwargs)


def kernel(x, wq, wk, wv, wo, sin, cos):
    in_maps = _host_prep(x, wq, wk, wv, wo, sin, cos)
    res = run_spmd(in_maps)
    out = np.stack([np.asarray(res.results[b]["out"], np.float32)
                    for b in range(N_CORES)], axis=0)
    return out
